# revision 9
# baseline (speedup 1.0000x reference)
"""CST airfoil decoder kernel for Trainium2 (Bass/Tile), 8-core data parallel.

Problem (hardcoded shapes): z (4096, 18) f32, x_coords (4096, 2048) f32
-> out (4096, 4096) f32 with out[:, 0::2] = x_coords, out[:, 1::2] = y.

y = C(x) * P_sel(x) + le_w * x * (1-x)^8.5 +/- te_h * x
  C(x)    = sqrt(clip(x, EPS)) * (1 - x)
  P_sel   = degree-7 polynomial, upper coeffs where j <= argmin_j(x) else lower
(The Bernstein-basis einsum of the reference is converted on the host to
monomial coefficients per row; selection between upper/lower happens on-device
per element with a prefix-min based first-argmin mask.)

Sharding: pure data parallel over the batch dim, 512 rows per NeuronCore.
"""

import math

import numpy as np

import concourse.bacc as bacc
import concourse.bass as bass
import concourse.mybir as mybir
from concourse.bass_utils import run_bass_kernel_spmd
from concourse.tile import TileContext

B, NZ = 4096, 18
N = 2048
N_CORES = 8
ROWS_PER_CORE = B // N_CORES          # 512
P = 128                               # partitions
TILES = ROWS_PER_CORE // P            # 4
EPS = 1e-8
NSC = 20                              # per-row scalar columns (19 used, padded)

F32 = mybir.dt.float32
Alu = mybir.AluOpType
Act = mybir.ActivationFunctionType


def _monomial_matrix() -> np.ndarray:
    """M[k, m]: coefficient of x^m in C(7,k) x^k (1-x)^(7-k)."""
    M = np.zeros((8, 8), dtype=np.float64)
    for k in range(8):
        c7k = math.comb(7, k)
        for m in range(k, 8):
            M[k, m] = c7k * math.comb(7 - k, m - k) * ((-1) ** (m - k))
    return M


def _host_scalars(z: np.ndarray) -> np.ndarray:
    """Pack per-row scalars: [aU(8) | aL(8) | le_w | 2*te_h | -te_h | pad]."""
    z64 = z.astype(np.float64)
    M = _monomial_matrix()
    aL = z64[:, 0:8] @ M
    aU = z64[:, 8:16] @ M
    le_w = z64[:, 16]
    te = z64[:, 17]                    # te_h = te / 2
    sc = np.zeros((B, NSC), dtype=np.float64)
    sc[:, 0:8] = aU
    sc[:, 8:16] = aL
    sc[:, 16] = le_w
    sc[:, 17] = te                     # 2 * te_h
    sc[:, 18] = -0.5 * te              # -te_h
    return sc.astype(np.float32)


def _build_program() -> bass.Bass:
    nc = bacc.Bacc("TRN2", debug=False, num_devices=N_CORES,
                   enable_partition_id=False)
    x_d = nc.dram_tensor("x", (ROWS_PER_CORE, N), F32, kind="ExternalInput")
    sc_d = nc.dram_tensor("sc", (ROWS_PER_CORE, NSC), F32, kind="ExternalInput")
    out_d = nc.dram_tensor("out", (ROWS_PER_CORE, 2 * N), F32,
                           kind="ExternalOutput")

    with TileContext(nc) as tc:
        with tc.tile_pool(name="io", bufs=2) as io_pool, \
             tc.tile_pool(name="scr", bufs=1) as scr:
            for t in range(TILES):
                r0 = t * P
                x = io_pool.tile([P, N], F32, tag="x")
                sc = io_pool.tile([P, NSC], F32, tag="sc")
                out = io_pool.tile([P, 2 * N], F32, tag="out")
                nc.sync.dma_start(out=x[:, :], in_=x_d.ap()[r0:r0 + P, :])
                nc.sync.dma_start(out=sc[:, :], in_=sc_d.ap()[r0:r0 + P, :])

                def col(i):
                    return sc[:, i:i + 1]

                # ---- mask: is_upper = (exclusive prefix min > row min) ----
                incl = scr.tile([P, N], F32, tag="incl")
                mask = scr.tile([P, N], F32, tag="mask")
                nc.vector.tensor_tensor_scan(
                    out=incl[:, :], data0=x[:, :], data1=x[:, :],
                    initial=2.0, op0=Alu.min, op1=Alu.min)
                nc.vector.tensor_scalar(
                    out=mask[:, 1:N], in0=incl[:, 0:N - 1],
                    scalar1=incl[:, N - 1:N], scalar2=None, op0=Alu.is_gt)
                nc.gpsimd.memset(mask[:, 0:1], 1.0)

                # ---- C(x) = sqrt(max(x, EPS)) * (1 - x) ----
                sqx = scr.tile([P, N], F32, tag="sqx")
                v = scr.tile([P, N], F32, tag="v")
                nc.gpsimd.tensor_scalar(out=sqx[:, :], in0=x[:, :],
                                        scalar1=EPS, scalar2=None, op0=Alu.max)
                nc.scalar.activation(out=sqx[:, :], in_=sqx[:, :], func=Act.Ln)
                nc.scalar.activation(out=sqx[:, :], in_=sqx[:, :], func=Act.Exp,
                                     scale=0.5)
                nc.gpsimd.tensor_scalar(out=v[:, :], in0=x[:, :], scalar1=-1.0,
                                        scalar2=1.0, op0=Alu.mult, op1=Alu.add)

                # ---- v85 = (1-x)^8.5 via exp(8.5 * ln(1-x)) ----
                v85 = scr.tile([P, N], F32, tag="v85")
                nc.scalar.activation(out=v85[:, :], in_=x[:, :], func=Act.Ln,
                                     scale=-1.0, bias=1.0)
                nc.scalar.activation(out=v85[:, :], in_=v85[:, :], func=Act.Exp,
                                     scale=8.5)

                # ---- powers of x (gpsimd) ----
                x2 = scr.tile([P, N], F32, tag="x2")
                x4 = scr.tile([P, N], F32, tag="x4")
                nc.gpsimd.tensor_tensor(out=x2[:, :], in0=x[:, :], in1=x[:, :],
                                        op=Alu.mult)
                nc.gpsimd.tensor_tensor(out=x4[:, :], in0=x2[:, :],
                                        in1=x2[:, :], op=Alu.mult)

                # ---- pair terms T_j = a[2j] + a[2j+1] * x  (ACT) ----
                TL = [scr.tile([P, N], F32, tag=f"TL{j}", name=f"TL{j}")
                      for j in range(4)]
                TU = [scr.tile([P, N], F32, tag=f"TU{j}", name=f"TU{j}")
                      for j in range(4)]
                for j in range(4):
                    nc.scalar.activation(out=TU[j][:, :], in_=x[:, :],
                                         func=Act.Identity,
                                         bias=col(2 * j), scale=col(2 * j + 1))
                    nc.scalar.activation(out=TL[j][:, :], in_=x[:, :],
                                         func=Act.Identity,
                                         bias=col(8 + 2 * j),
                                         scale=col(8 + 2 * j + 1))
                # select upper where mask!=0 (in place into TL)
                mask_u32 = mask[:, :].bitcast(mybir.dt.uint32)
                for j in range(4):
                    nc.vector.copy_predicated(out=TL[j][:, :], mask=mask_u32,
                                              data=TU[j][:, :])

                # ---- Estrin: P = (T0 + x2*T1) + x4*(T2 + x2*T3) ----
                m1, m2, m3 = TU[0], TU[1], TU[2]
                nc.vector.tensor_mul(out=m1[:, :], in0=x2[:, :], in1=TL[1][:, :])
                nc.vector.tensor_add(out=TL[0][:, :], in0=TL[0][:, :],
                                     in1=m1[:, :])
                nc.vector.tensor_mul(out=m2[:, :], in0=x2[:, :], in1=TL[3][:, :])
                nc.vector.tensor_add(out=TL[2][:, :], in0=TL[2][:, :],
                                     in1=m2[:, :])
                nc.vector.tensor_mul(out=m3[:, :], in0=x4[:, :], in1=TL[2][:, :])
                nc.vector.tensor_add(out=TL[0][:, :], in0=TL[0][:, :],
                                     in1=m3[:, :])

                # ---- y = C * P + x * (le_w * v85 + (2*te_h*mask - te_h)) ----
                nc.vector.tensor_mul(out=v[:, :], in0=sqx[:, :], in1=v[:, :])
                nc.vector.tensor_mul(out=TL[0][:, :], in0=v[:, :],
                                     in1=TL[0][:, :])
                inner = TU[3]
                nc.scalar.activation(out=inner[:, :], in_=mask[:, :],
                                     func=Act.Identity,
                                     bias=col(18), scale=col(17))
                nc.scalar.activation(out=v85[:, :], in_=v85[:, :],
                                     func=Act.Identity,
                                     bias=0.0, scale=col(16))
                nc.gpsimd.tensor_tensor(out=inner[:, :], in0=v85[:, :],
                                        in1=inner[:, :], op=Alu.add)
                xin = TU[1]
                nc.vector.tensor_mul(out=xin[:, :], in0=x[:, :],
                                     in1=inner[:, :])

                out3 = out[:, :].rearrange("p (n two) -> p n two", two=2)
                nc.gpsimd.tensor_copy(out=out3[:, :, 0:1], in_=x[:, :])
                nc.vector.tensor_add(out=out3[:, :, 1:2], in0=TL[0][:, :],
                                     in1=xin[:, :])

                nc.sync.dma_start(out=out_d.ap()[r0:r0 + P, :], in_=out[:, :])
    nc.compile()
    return nc


_PROGRAM: bass.Bass | None = None


def _program() -> bass.Bass:
    global _PROGRAM
    if _PROGRAM is None:
        _PROGRAM = _build_program()
    return _PROGRAM


def kernel(z, x_coords, _run_kwargs: dict | None = None):
    z = np.asarray(z, dtype=np.float32)
    x_coords = np.ascontiguousarray(np.asarray(x_coords, dtype=np.float32))
    assert z.shape == (B, NZ) and x_coords.shape == (B, N)

    sc = _host_scalars(z)
    in_maps = []
    for c in range(N_CORES):
        r = slice(c * ROWS_PER_CORE, (c + 1) * ROWS_PER_CORE)
        in_maps.append({"x": np.ascontiguousarray(x_coords[r]),
                        "sc": np.ascontiguousarray(sc[r])})

    res = run_bass_kernel_spmd(_program(), in_maps,
                               core_ids=list(range(N_CORES)),
                               **(_run_kwargs or {}))
    out = np.concatenate([r["out"] for r in res.results], axis=0)
    if _run_kwargs:
        kernel.last_results = res
    return out
